# revision 3
# baseline (speedup 1.0000x reference)
"""Trainium2 Bass kernel for nn_DMGCNLayer (GNN message passing layer).

Strategy (graph/data parallel over 8 NeuronCores):
  - Edges are bucketed by dst node range (6250 nodes per core) so each core
    produces a disjoint slice of the output -> no cross-core reduction.
  - Within a core, edges are ordered by 128-node dst window with uniform
    (max-over-cores) per-bucket budgets so that all 8 cores execute one
    identical SPMD program; shortfall is padded with self-neutralizing
    edges (their window-relative dst is 200, which matches no one-hot column).
  - The run is tunnel-transfer-bound (axon tunnel ~55 MB/s), so all per-edge
    payloads travel as 1-byte types: h[src] and eh stream as float8_e3m4
    (converted to bf16 on device), window-relative dst as uint8. The h-window
    table also travels fp8. Output aggregates return as bf16; the exact fp32
    residual (+h) is added on the host.
  - h[dst] and the segment-sum are realized as one-hot matmuls on the tensor
    engine (edges are sorted by dst window), accumulating in fp32 PSUM.
  - The message MLPs run in transposed-activation form in bf16 with folded
    weights: m2 = relu(eh@(0.8 W_e1) + (hs*hd)@(0.2 W_ue@W_e1) + b_e1)@W_e2.
  - The PJRT dispatch path is cached: inputs are prepared directly into
    concatenated global arrays, the shard_map jit is built once, and output
    zero-buffers are created device-side (nothing extra over the tunnel).
"""

import time
from contextlib import ExitStack
from dataclasses import dataclass

import numpy as np
import ml_dtypes

import concourse.bass as bass
import concourse.bacc as bacc
import concourse.mybir as mybir
import concourse.tile as tile

BF16 = ml_dtypes.bfloat16
F8 = ml_dtypes.float8_e3m4
PADVAL = 200  # window-relative dst for pad edges; matches no iota column


@dataclass(frozen=True)
class Cfg:
    N: int = 50000
    E: int = 800000
    DN: int = 64
    H: int = 128
    NC: int = 8          # cores
    ST: int = 1024       # supertile (edges per pipeline step)

    @property
    def NR(self):  # nodes per core
        return self.N // self.NC

    @property
    def NW(self):  # 128-node windows per core
        return -(-self.NR // 128)


CFG_FULL = Cfg()


# --------------------------------------------------------------------------
# planning (uniform across cores)
# --------------------------------------------------------------------------

@dataclass
class Plan:
    budg: np.ndarray      # [NW] edge budget per window, 128-mult
    pos0: np.ndarray      # [NW] start position of each bucket
    ET: int               # total positions per core (multiple of ST)
    wchunk: np.ndarray    # [ET//128] window id of each 128-chunk
    first_chunk: np.ndarray  # [ET//128] bool: first chunk of its window block
    last_chunk: np.ndarray   # [ET//128] bool: last chunk of its window block


def _make_plan(cfg: Cfg, src: np.ndarray, dst: np.ndarray) -> Plan:
    NR, NW = cfg.NR, cfg.NW
    core = dst // NR
    win = (dst % NR) // 128

    counts = np.zeros((cfg.NC, NW), np.int64)
    np.add.at(counts, (core, win), 1)
    budg = counts.max(axis=0)
    budg = ((budg + 127) // 128) * 128
    # pad the total to a multiple of ST (grow the last window's budget with
    # pure-pad chunks; pads neutralize themselves via the one-hot miss)
    budg[NW - 1] += (-budg.sum()) % cfg.ST

    pos0 = np.zeros(NW, np.int64)
    off = 0
    for w in range(NW):
        pos0[w] = off
        off += budg[w]
    ET = int(off)
    assert ET % cfg.ST == 0

    nch = ET // 128
    wchunk = np.zeros(nch, np.int64)
    first_chunk = np.zeros(nch, bool)
    last_chunk = np.zeros(nch, bool)
    for w in range(NW):
        c0 = int(pos0[w]) // 128
        c1 = c0 + int(budg[w]) // 128
        wchunk[c0:c1] = w
        first_chunk[c0] = True
        last_chunk[c1 - 1] = True
    return Plan(budg, pos0, ET, wchunk, first_chunk, last_chunk)


# --------------------------------------------------------------------------
# host-side input preparation (writes straight into global concat arrays)
# --------------------------------------------------------------------------

def _in_shapes(cfg: Cfg, plan: Plan):
    """name -> (per-core shape, numpy dtype), in a fixed order."""
    ET, NW, H = plan.ET, cfg.NW, cfg.H
    return {
        "eh_t": ((64, ET), F8),
        "hs_t": ((64, ET), F8),
        "wrel_col": ((128, ET // 128), np.uint8),
        "wrel_row": ((1, ET), np.uint8),
        "hwin": ((128, NW * 64), F8),
        "wzp": ((128, H), BF16),
        "we2": ((H, H), BF16),
        "wcomb": ((H, 64), BF16),
        "be1": ((H, 1), np.float32),
        "iota_t": ((128, 128), BF16),
        "iota_c": ((128, 1), np.float32),
        "ones1": ((1, 128), BF16),
        "wn1": ((64, H), BF16),
        "wn2": ((H, H), BF16),
        "bn1": ((H, 1), np.float32),
    }


def _prep(cfg: Cfg, inputs: dict, plan: Plan):
    h = np.asarray(inputs["h"], np.float32)
    eh = np.asarray(inputs["eh"], np.float32)
    src = np.asarray(inputs["src"]).astype(np.int64)
    dst = np.asarray(inputs["dst"]).astype(np.int64)
    W_node1 = np.asarray(inputs["W_node1"], np.float32)
    b_node1 = np.asarray(inputs["b_node1"], np.float32)
    W_node2 = np.asarray(inputs["W_node2"], np.float32)
    W_edge1 = np.asarray(inputs["W_edge1"], np.float32)
    b_edge1 = np.asarray(inputs["b_edge1"], np.float32)
    W_edge2 = np.asarray(inputs["W_edge2"], np.float32)
    W_comb = np.asarray(inputs["W_comb"], np.float32)
    W_ue = np.asarray(inputs["W_ue"], np.float32)

    NR, NW, ET, NC = cfg.NR, cfg.NW, plan.ET, cfg.NC
    shapes = _in_shapes(cfg, plan)
    glo = {name: np.zeros((NC * shp[0], *shp[1:]), dt)
           for name, (shp, dt) in shapes.items()}

    def per_core(name, k):
        d0 = shapes[name][0][0]
        return glo[name][k * d0:(k + 1) * d0]

    # folded weights (replicated to every core)
    A = (0.8 * W_edge1).astype(BF16)                 # [64, H]
    W_ue1 = (0.2 * (W_ue @ W_edge1)).astype(BF16)    # [64, H]
    wzp = np.concatenate([W_ue1, A], axis=0)         # [128, H]; rows 0:64 act on p
    iota_t = np.broadcast_to(np.arange(128, dtype=np.float32),
                             (128, 128)).astype(BF16)
    for k in range(NC):
        per_core("wzp", k)[:] = wzp
        per_core("we2", k)[:] = W_edge2.astype(BF16)
        per_core("wcomb", k)[:] = W_comb.astype(BF16)
        per_core("be1", k)[:] = b_edge1.reshape(cfg.H, 1)
        per_core("iota_t", k)[:] = iota_t
        per_core("iota_c", k)[:] = np.arange(128, dtype=np.float32).reshape(128, 1)
        per_core("ones1", k)[:] = np.ones((1, 128), BF16)
        per_core("wn1", k)[:] = W_node1.astype(BF16)
        per_core("wn2", k)[:] = W_node2.astype(BF16)
        per_core("bn1", k)[:] = b_node1.reshape(cfg.H, 1)

    hs_f8 = h.astype(F8)
    eh_f8 = eh.astype(F8)

    core = dst // NR
    win = (dst % NR) // 128

    for k in range(NC):
        # fill positions: bucket edges then pads
        perm = np.full(ET, -1, np.int64)
        ek = np.nonzero(core == k)[0]
        key = win[ek]
        order = np.argsort(key, kind="stable")
        ek = ek[order]
        key = key[order]
        starts = plan.pos0[key]
        changes = np.r_[True, key[1:] != key[:-1]]
        grp_start_idx = np.r_[0, np.nonzero(changes)[0][1:]]
        grp_id = np.cumsum(changes) - 1
        rank = np.arange(len(ek)) - grp_start_idx[grp_id]
        pos = starts + rank
        perm[pos] = ek

        valid = perm >= 0
        pe = perm[valid]

        per_core("eh_t", k)[:, valid] = eh_f8[pe].T
        per_core("hs_t", k)[:, valid] = hs_f8[src[pe]].T

        wrel = np.full(ET, PADVAL, np.uint8)
        wrel[valid] = (dst[pe] - k * NR - win[pe] * 128).astype(np.uint8)
        per_core("wrel_col", k)[:] = wrel.reshape(ET // 128, 128).T
        per_core("wrel_row", k)[:] = wrel.reshape(1, ET)

        hwin_k = per_core("hwin", k)
        hk = h[k * NR:(k + 1) * NR].astype(F8)            # [NR, 64]
        for w in range(NW):
            rows = hk[w * 128:(w + 1) * 128]
            hwin_k[:rows.shape[0], w * 64:w * 64 + 64] = rows

    ctx = {"h": h}
    return glo, ctx


# --------------------------------------------------------------------------
# device program
# --------------------------------------------------------------------------

def _build(cfg: Cfg, plan: Plan) -> bacc.Bacc:
    ET, NW = plan.ET, cfg.NW
    f32 = mybir.dt.float32
    bf16 = mybir.dt.bfloat16
    f8 = mybir.dt.float8e3
    u8 = mybir.dt.uint8

    nc = bacc.Bacc("TRN2", target_bir_lowering=False, debug=False,
                   enable_asserts=False)

    d_eh = nc.dram_tensor("eh_t", [64, ET], f8, kind="ExternalInput").ap()
    d_hst = nc.dram_tensor("hs_t", [64, ET], f8, kind="ExternalInput").ap()
    d_wrc = nc.dram_tensor("wrel_col", [128, ET // 128], u8, kind="ExternalInput").ap()
    d_wrr = nc.dram_tensor("wrel_row", [1, ET], u8, kind="ExternalInput").ap()
    d_hwin = nc.dram_tensor("hwin", [128, NW * 64], f8, kind="ExternalInput").ap()
    d_wzp = nc.dram_tensor("wzp", [128, cfg.H], bf16, kind="ExternalInput").ap()
    d_we2 = nc.dram_tensor("we2", [cfg.H, cfg.H], bf16, kind="ExternalInput").ap()
    d_wcomb = nc.dram_tensor("wcomb", [cfg.H, 64], bf16, kind="ExternalInput").ap()
    d_be1 = nc.dram_tensor("be1", [cfg.H, 1], f32, kind="ExternalInput").ap()
    d_iota_t = nc.dram_tensor("iota_t", [128, 128], bf16, kind="ExternalInput").ap()
    d_iota_c = nc.dram_tensor("iota_c", [128, 1], f32, kind="ExternalInput").ap()
    d_ones1 = nc.dram_tensor("ones1", [1, 128], bf16, kind="ExternalInput").ap()
    d_wn1 = nc.dram_tensor("wn1", [64, cfg.H], bf16, kind="ExternalInput").ap()
    d_wn2 = nc.dram_tensor("wn2", [cfg.H, cfg.H], bf16, kind="ExternalInput").ap()
    d_bn1 = nc.dram_tensor("bn1", [cfg.H, 1], f32, kind="ExternalInput").ap()
    d_agg = nc.dram_tensor("agg", [128, NW * 64], bf16, kind="ExternalOutput").ap()

    eq = mybir.AluOpType.is_equal
    mul = mybir.AluOpType.mult
    add = mybir.AluOpType.add
    Relu = mybir.ActivationFunctionType.Relu
    Tanh = mybir.ActivationFunctionType.Tanh

    NSTEP = ET // cfg.ST

    with tile.TileContext(nc) as tc, ExitStack() as ctx:
        con = ctx.enter_context(tc.tile_pool(name="const", bufs=1))
        sb = ctx.enter_context(tc.tile_pool(name="sb", bufs=2))
        sohp = ctx.enter_context(tc.tile_pool(name="soh", bufs=12))
        gpool = ctx.enter_context(tc.tile_pool(name="gbuf", bufs=2))
        pers = ctx.enter_context(tc.tile_pool(name="pers", bufs=1))
        ps_a = ctx.enter_context(tc.tile_pool(name="ps_a", bufs=1, space="PSUM"))
        ps_b = ctx.enter_context(tc.tile_pool(name="ps_b", bufs=1, space="PSUM"))
        ps_hd = ctx.enter_context(tc.tile_pool(name="ps_hd", bufs=1, space="PSUM"))
        ps_bc = ctx.enter_context(tc.tile_pool(name="ps_bc", bufs=1, space="PSUM"))
        ps_mn = ctx.enter_context(tc.tile_pool(name="ps_mn", bufs=1, space="PSUM"))
        ps_ag = ctx.enter_context(tc.tile_pool(name="ps_ag", bufs=1, space="PSUM"))

        def load_const(tag, dram_ap, shape, dtype):
            t_ = con.tile(shape, dtype, tag=tag)
            nc.sync.dma_start(out=t_[:], in_=dram_ap)
            return t_

        c_wzp = load_const("wzp", d_wzp, [128, cfg.H], bf16)
        c_we2 = load_const("we2", d_we2, [cfg.H, cfg.H], bf16)
        c_wcomb = load_const("wcomb", d_wcomb, [cfg.H, 64], bf16)
        c_be1 = load_const("be1", d_be1, [cfg.H, 1], f32)
        c_iota_t = load_const("iota_t", d_iota_t, [128, 128], bf16)
        c_iota_c = load_const("iota_c", d_iota_c, [128, 1], f32)
        c_ones1 = load_const("ones1", d_ones1, [1, 128], bf16)
        c_hwin = load_const("hwin", d_hwin, [128, NW * 64], f8)
        c_wrc8 = load_const("wrc8", d_wrc, [128, ET // 128], u8)
        c_wn1 = load_const("wn1", d_wn1, [64, cfg.H], bf16)
        c_wn2 = load_const("wn2", d_wn2, [cfg.H, cfg.H], bf16)
        c_bn1 = load_const("bn1", d_bn1, [cfg.H, 1], f32)

        # one-time uint8 -> f32 conversion of the per-chunk dst columns
        c_wrc = con.tile([128, ET // 128], f32, tag="wrc")
        nc.vector.tensor_copy(out=c_wrc[:], in_=c_wrc8[:])

        agg_sb = pers.tile([128, NW * 64], bf16)
        aggp = ps_ag.tile([128, 8, 64], f32)  # rotating window accumulators

        for t in range(NSTEP):
            hsq = gpool.tile([64, cfg.ST], f8, tag="hsq")
            nc.sync.dma_start(out=hsq[:],
                              in_=d_hst[:, t * cfg.ST:(t + 1) * cfg.ST])
            hsb = gpool.tile([64, cfg.ST], bf16, tag="hsb")
            nc.vector.tensor_copy(out=hsb[:], in_=hsq[:])

            # per-edge MLP1: m1 = relu(hs@Wn1 + bn1)@Wn2, in transposed form
            z1 = ps_a.tile([128, cfg.ST], f32, tag="za")
            for hhalf in range(cfg.ST // 512):
                cl0 = hhalf * 512
                nc.tensor.matmul(z1[:, cl0:cl0 + 512], c_wn1[:],
                                 hsb[:, cl0:cl0 + 512],
                                 start=True, stop=True)
            r1 = sb.tile([128, cfg.ST], bf16, tag="r1")
            nc.vector.tensor_scalar(r1[:], z1[:], c_bn1[:, 0:1], 0.0,
                                    mybir.AluOpType.add, mybir.AluOpType.max)
            m1p = ps_b.tile([128, cfg.ST], f32, tag="zb")
            for hhalf in range(cfg.ST // 512):
                cl0 = hhalf * 512
                nc.tensor.matmul(m1p[:, cl0:cl0 + 512], c_wn2[:],
                                 r1[:, cl0:cl0 + 512], start=True, stop=True)
            m1sb = sb.tile([128, cfg.ST], bf16, tag="m1sb")
            nc.vector.tensor_copy(out=m1sb[:], in_=m1p[:])

            stack = sb.tile([128, cfg.ST], bf16, tag="stack")
            ehq = gpool.tile([64, cfg.ST], f8, tag="ehq")
            nc.sync.dma_start(out=ehq[:],
                              in_=d_eh[:, t * cfg.ST:(t + 1) * cfg.ST])
            nc.scalar.activation(stack[64:128, :], ehq[:],
                                 mybir.ActivationFunctionType.Copy)
            wrr8 = sb.tile([1, cfg.ST], u8, tag="wrr8")
            nc.sync.dma_start(out=wrr8[:], in_=d_wrr[:, t * cfg.ST:(t + 1) * cfg.ST])
            wrr = sb.tile([1, cfg.ST], bf16, tag="wrr")
            nc.vector.tensor_copy(out=wrr[:], in_=wrr8[:])

            # per-128-chunk segment one-hot [edge, node-in-window]
            seg_ohs = []
            for j in range(cfg.ST // 128):
                c = t * (cfg.ST // 128) + j
                so = sohp.tile([128, 128], bf16, tag="soh")
                nc.vector.tensor_scalar(so[:], c_iota_t[:], c_wrc[:, c:c + 1],
                                        None, eq)
                seg_ohs.append(so)

            # hd via one-hot matmul, in 512-col halves
            for hhalf in range(cfg.ST // 512):
                cl0 = hhalf * 512
                bc = ps_bc.tile([128, 512], f32, tag="bc")
                nc.tensor.matmul(bc[:], c_ones1[:],
                                 wrr[:, cl0:cl0 + 512], start=True, stop=True)
                ohT = sb.tile([128, 512], bf16, tag="ohT")
                nc.vector.tensor_scalar(ohT[:], bc[:], c_iota_c[:], None, eq)
                hd = ps_hd.tile([64, 512], f32, tag="hd")
                # window-parts inside this half (chunks are window-pure)
                j0 = cl0 // 128
                parts = []
                for j in range(j0, j0 + 4):
                    c = t * (cfg.ST // 128) + j
                    w = int(plan.wchunk[c])
                    if parts and parts[-1][2] == w:
                        parts[-1][1] += 128
                    else:
                        parts.append([j * 128 - cl0, 128, w])
                for (o, wd, w) in parts:
                    nc.tensor.matmul(hd[:, o:o + wd],
                                     c_hwin[:, w * 64:(w + 1) * 64],
                                     ohT[:, o:o + wd], start=True, stop=True)
                # p = hs * hd  -> stack partitions 0:64
                nc.vector.tensor_tensor(
                    out=stack[0:64, cl0:cl0 + 512],
                    in0=hsb[:, cl0:cl0 + 512],
                    in1=hd[:, :], op=mul)

            z = ps_a.tile([128, cfg.ST], f32, tag="za")
            for hhalf in range(cfg.ST // 512):
                cl0 = hhalf * 512
                nc.tensor.matmul(z[:, cl0:cl0 + 512], c_wzp[:],
                                 stack[:, cl0:cl0 + 512], start=True, stop=True)
            rz = sb.tile([128, cfg.ST], bf16, tag="rz")
            nc.scalar.activation(rz[:], z[:], Relu, bias=c_be1[:, 0:1])

            m2 = ps_b.tile([128, cfg.ST], f32, tag="zb")
            for hhalf in range(cfg.ST // 512):
                cl0 = hhalf * 512
                nc.tensor.matmul(m2[:, cl0:cl0 + 512], c_we2[:],
                                 rz[:, cl0:cl0 + 512], start=True, stop=True)

            m2c = sb.tile([128, cfg.ST], bf16, tag="m2c")
            nc.scalar.activation(m2c[:], m2[:],
                                 mybir.ActivationFunctionType.Copy)
            q = sb.tile([128, cfg.ST], bf16, tag="q")
            nc.gpsimd.tensor_tensor(out=q[:, :], in0=m1sb[:, :],
                                    in1=m2c[:, :], op=mul)

            mnt = ps_mn.tile([128, cfg.ST // 128, 64], f32, tag="mnt")
            for j in range(cfg.ST // 128):
                nc.tensor.matmul(mnt[:, j, :], q[:, j * 128:(j + 1) * 128],
                                 c_wcomb[:], start=True, stop=True)
            msb = sb.tile([128, cfg.ST // 128, 64], bf16, tag="msb")
            nc.scalar.activation(msb[:], mnt[:], Tanh)

            for j in range(cfg.ST // 128):
                c = t * (cfg.ST // 128) + j
                w = int(plan.wchunk[c])
                first = bool(plan.first_chunk[c])
                last = bool(plan.last_chunk[c])
                slot = w % 8
                nc.tensor.matmul(aggp[:, slot, :], seg_ohs[j][:],
                                 msb[:, j, :], start=first, stop=last)
                if last:
                    nc.vector.tensor_copy(out=agg_sb[:, w * 64:(w + 1) * 64],
                                          in_=aggp[:, slot, :])

        nc.sync.dma_start(out=d_agg, in_=agg_sb[:])

    nc.compile()
    return nc


# --------------------------------------------------------------------------
# cached PJRT executor (jit built once; zeros created device-side)
# --------------------------------------------------------------------------

class Exec:
    def __init__(self, nc_bass, n_cores: int):
        import jax
        import numpy as _np
        from jax.sharding import Mesh, NamedSharding, PartitionSpec
        from jax.experimental.shard_map import shard_map
        from concourse import bass2jax

        bass2jax.install_neuronx_cc_hook()
        self.nc = nc_bass
        self.n_cores = n_cores
        assert nc_bass.dbg_addr is None or not nc_bass.dbg_callbacks
        partition_name = (nc_bass.partition_id_tensor.name
                          if nc_bass.partition_id_tensor else None)

        in_names, out_names, out_avals, zero_shapes = [], [], [], []
        for alloc in nc_bass.m.functions[0].allocations:
            if not isinstance(alloc, mybir.MemoryLocationSet):
                continue
            name = alloc.memorylocations[0].name
            if alloc.kind == "ExternalInput":
                if name != partition_name:
                    in_names.append(name)
            elif alloc.kind == "ExternalOutput":
                out_names.append(name)
                shape = tuple(alloc.tensor_shape)
                dtype = mybir.dt.np(alloc.dtype)
                out_avals.append(jax.core.ShapedArray(shape, dtype))
                zero_shapes.append((shape, dtype))
        self.in_names = in_names
        self.out_names = out_names
        n_params = len(in_names)
        n_outs = len(out_avals)
        all_in_names = list(in_names) + list(out_names)
        if partition_name is not None:
            all_in_names.append(partition_name)
        donate = tuple(range(n_params, n_params + n_outs))
        self.zero_shapes = zero_shapes

        def _body(*args):
            operands = list(args)
            if partition_name is not None:
                operands.append(bass2jax.partition_id_tensor())
            outs = bass2jax._bass_exec_p.bind(
                *operands,
                out_avals=tuple(out_avals),
                in_names=tuple(all_in_names),
                out_names=tuple(out_names),
                lowering_input_output_aliases=(),
                sim_require_finite=True,
                sim_require_nnan=True,
                nc=nc_bass,
            )
            return tuple(outs)

        devices = jax.devices()[:n_cores]
        assert len(devices) == n_cores
        mesh = Mesh(_np.asarray(devices), ("core",))
        in_specs = (PartitionSpec("core"),) * (n_params + n_outs)
        out_specs = (PartitionSpec("core"),) * len(out_names)
        self.sharded = jax.jit(
            shard_map(_body, mesh=mesh, in_specs=in_specs,
                      out_specs=out_specs, check_rep=False),
            donate_argnums=donate, keep_unused=True,
        )
        self.sharding = NamedSharding(mesh, PartitionSpec("core"))
        import jax.numpy as jnp
        self._jnp = jnp

    def __call__(self, global_in: dict) -> dict:
        """global_in: name -> concatenated [n_cores*d0, ...] host array.
        Returns name -> concatenated output array (numpy)."""
        jnp = self._jnp
        args = [global_in[n] for n in self.in_names]
        zeros = [jnp.zeros((self.n_cores * s[0], *s[1:]), d,
                           device=self.sharding) for (s, d) in self.zero_shapes]
        outs = self.sharded(*args, *zeros)
        return {name: np.asarray(o) for name, o in zip(self.out_names, outs)}


# --------------------------------------------------------------------------
# entry points
# --------------------------------------------------------------------------

def _assemble(cfg: Cfg, agg_global: np.ndarray, ctx):
    h = ctx["h"]
    out = np.empty((cfg.N, cfg.DN), np.float32)
    for k in range(cfg.NC):
        agg = agg_global[k * 128:(k + 1) * 128].astype(np.float32)
        agg = agg.reshape(128, cfg.NW, 64).transpose(1, 0, 2).reshape(cfg.NW * 128, 64)
        out[k * cfg.NR:(k + 1) * cfg.NR] = agg[:cfg.NR] + h[k * cfg.NR:(k + 1) * cfg.NR]
    return out


def run_pipeline(cfg: Cfg, inputs: dict, backend: str = "hw", reps: int = 1):
    """Returns (output, steady_state_wall_seconds_or_None)."""
    src = np.asarray(inputs["src"]).astype(np.int64)
    dst = np.asarray(inputs["dst"]).astype(np.int64)
    plan = _make_plan(cfg, src, dst)
    glo, ctx = _prep(cfg, inputs, plan)
    nc = _build(cfg, plan)
    if backend == "sim":
        from concourse.bass_interp import CoreSim
        shapes = _in_shapes(cfg, plan)
        aggs = []
        for k in range(cfg.NC):
            sim = CoreSim(nc, trace=False)
            for name, (shp, _dt) in shapes.items():
                d0 = shp[0]
                sim.tensor(name)[:] = glo[name][k * d0:(k + 1) * d0]
            sim.simulate()
            aggs.append(np.array(sim.tensor("agg")))
        return _assemble(cfg, np.concatenate(aggs, axis=0), ctx), None
    ex = Exec(nc, cfg.NC)
    res = ex(glo)  # first call traces + compiles
    dt = None
    for _ in range(max(0, reps - 1)):
        t0 = time.time()
        res = ex(glo)
        out = _assemble(cfg, res["agg"], ctx)
        dt = time.time() - t0
    out = _assemble(cfg, res["agg"], ctx)
    return out, dt


def kernel(**inputs) -> np.ndarray:
    out, _ = run_pipeline(CFG_FULL, inputs, backend="hw")
    return out


if __name__ == "__main__":
    # smoke test at small scale on the simulator
    cfg = Cfg(N=2048, E=8192, NC=2, ST=1024)
    rng = np.random.default_rng(0)
    inputs = {
        "h": rng.standard_normal((cfg.N, 64)).astype(np.float32),
        "eh": rng.standard_normal((cfg.E, 64)).astype(np.float32),
        "W_node1": (rng.standard_normal((64, 128)) * 0.05).astype(np.float32),
        "b_node1": (rng.standard_normal((128,)) * 0.05).astype(np.float32),
        "W_node2": (rng.standard_normal((128, 128)) * 0.05).astype(np.float32),
        "W_edge1": (rng.standard_normal((64, 128)) * 0.05).astype(np.float32),
        "b_edge1": (rng.standard_normal((128,)) * 0.05).astype(np.float32),
        "W_edge2": (rng.standard_normal((128, 128)) * 0.05).astype(np.float32),
        "W_comb": (rng.standard_normal((128, 64)) * 0.05).astype(np.float32),
        "W_ue": (rng.standard_normal((64, 64)) * 0.05).astype(np.float32),
        "src": rng.integers(0, cfg.N, cfg.E).astype(np.int32),
        "dst": rng.integers(0, cfg.N, cfg.E).astype(np.int32),
    }
    h, eh = inputs["h"], inputs["eh"]
    hs, hd = h[inputs["src"]], h[inputs["dst"]]
    eh_new = 0.8 * eh + 0.2 * ((hs * hd) @ inputs["W_ue"])
    m1 = np.maximum(hs @ inputs["W_node1"] + inputs["b_node1"], 0) @ inputs["W_node2"]
    m2 = np.maximum(eh_new @ inputs["W_edge1"] + inputs["b_edge1"], 0) @ inputs["W_edge2"]
    m = np.tanh((m1 * m2) @ inputs["W_comb"])
    agg = np.zeros((cfg.N, 64), np.float32)
    np.add.at(agg, inputs["dst"], m)
    expected = agg + h

    out, _ = run_pipeline(cfg, inputs, backend="sim")
    err = np.abs(out - expected)
    rel = np.abs(err).max() / np.abs(expected).max()
    print("max abs err:", err.max(), " rel(absmax):", rel)
    print("mean abs err:", err.mean())
    assert rel < 2e-2, "accuracy failure"
    print("SIM OK")


# revision 25
# speedup vs baseline: 1.7761x; 1.7761x over previous
"""Trainium2 Bass kernel for nn_DMGCNLayer (GNN message passing layer).

Strategy (graph/data parallel over 8 NeuronCores):
  - Edges are bucketed by dst node range (6250 nodes per core) so each core
    produces a disjoint slice of the output -> no cross-core reduction.
  - Within a core, edges are ordered by 128-node dst window with uniform
    (max-over-cores) per-bucket budgets so that all 8 cores execute one
    identical SPMD program; shortfall is padded with self-neutralizing
    edges (their window-relative dst is 200, which matches no one-hot column).
  - The run is tunnel-transfer-bound (axon tunnel ~55 MB/s), so all per-edge
    payloads travel as 1-byte types: h[src] and eh stream as float8_e3m4
    (converted to bf16 on device), window-relative dst as uint8. The h-window
    table also travels fp8. Output aggregates return as bf16; the exact fp32
    residual (+h) is added on the host.
  - h[dst] and the segment-sum are realized as one-hot matmuls on the tensor
    engine (edges are sorted by dst window), accumulating in fp32 PSUM.
  - The message MLPs run in transposed-activation form in bf16 with folded
    weights: m2 = relu(eh@(0.8 W_e1) + (hs*hd)@(0.2 W_ue@W_e1) + b_e1)@W_e2.
  - The PJRT dispatch path is cached: inputs are prepared directly into
    concatenated global arrays, the shard_map jit is built once, and output
    zero-buffers are created device-side (nothing extra over the tunnel).
"""

import time
from contextlib import ExitStack
from dataclasses import dataclass

import numpy as np
import ml_dtypes

import concourse.bass as bass
import concourse.bacc as bacc
import concourse.mybir as mybir
import concourse.tile as tile

BF16 = ml_dtypes.bfloat16
F8 = ml_dtypes.float8_e3m4
PADVAL = 200  # window-relative dst for pad edges; matches no iota column


@dataclass(frozen=True)
class Cfg:
    N: int = 50000
    E: int = 800000
    DN: int = 64
    H: int = 128
    NC: int = 8          # cores
    ST: int = 1024       # supertile (edges per pipeline step)
    q4_eh: bool = False  # ship eh as packed int4 (vs fp8)
    q4_hs: bool = False  # ship h[src] as packed int4 (vs fp8)
    q4_clip: float = 2.65

    @property
    def q4_step(self):
        return 2 * self.q4_clip / 15

    @property
    def NR(self):  # nodes per core
        return self.N // self.NC

    @property
    def NW(self):  # 128-node windows per core
        return -(-self.NR // 128)


CFG_FULL = Cfg(q4_eh=True, q4_hs=True)


# --------------------------------------------------------------------------
# planning (uniform across cores)
# --------------------------------------------------------------------------

@dataclass
class Plan:
    budg: np.ndarray      # [NW] edge budget per window, 128-mult
    pos0: np.ndarray      # [NW] start position of each bucket
    ET: int               # total positions per core (multiple of ST)
    wchunk: np.ndarray    # [ET//128] window id of each 128-chunk
    first_chunk: np.ndarray  # [ET//128] bool: first chunk of its window block
    last_chunk: np.ndarray   # [ET//128] bool: last chunk of its window block


def _make_plan(cfg: Cfg, src: np.ndarray, dst: np.ndarray) -> Plan:
    NR, NW = cfg.NR, cfg.NW
    core = dst // NR
    win = (dst % NR) // 128

    counts = np.zeros((cfg.NC, NW), np.int64)
    np.add.at(counts, (core, win), 1)
    budg = counts.max(axis=0)
    budg = ((budg + 127) // 128) * 128
    # pad the total to a multiple of ST (grow the last window's budget with
    # pure-pad chunks; pads neutralize themselves via the one-hot miss)
    budg[NW - 1] += (-budg.sum()) % cfg.ST

    pos0 = np.zeros(NW, np.int64)
    off = 0
    for w in range(NW):
        pos0[w] = off
        off += budg[w]
    ET = int(off)
    assert ET % cfg.ST == 0

    nch = ET // 128
    wchunk = np.zeros(nch, np.int64)
    first_chunk = np.zeros(nch, bool)
    last_chunk = np.zeros(nch, bool)
    for w in range(NW):
        c0 = int(pos0[w]) // 128
        c1 = c0 + int(budg[w]) // 128
        wchunk[c0:c1] = w
        first_chunk[c0] = True
        last_chunk[c1 - 1] = True
    return Plan(budg, pos0, ET, wchunk, first_chunk, last_chunk)


# --------------------------------------------------------------------------
# host-side input preparation (writes straight into global concat arrays)
# --------------------------------------------------------------------------

def _in_shapes(cfg: Cfg, plan: Plan):
    """name -> (per-core shape, numpy dtype), in a fixed order."""
    ET, NW, H = plan.ET, cfg.NW, cfg.H
    return {
        "eh_t": ((64, ET // 2), np.uint8) if cfg.q4_eh else ((64, ET), F8),
        "hs_t": ((64, ET // 2), np.uint8) if cfg.q4_hs else ((64, ET), F8),
        "wrel_col": ((128, ET // 128), np.uint8),
        "hwin": ((128, NW * 32), np.uint8),
        "wzp": ((128, H), BF16),
        "we2": ((H, H), BF16),
        "wcomb": ((H, 64), BF16),
        "be1": ((H, 1), np.float32),
        "iota_t": ((128, 128), BF16),
        "wn1": ((64, H), BF16),
        "wn2": ((H, H), BF16),
        "bn1": ((H, 1), np.float32),
    }


def _q4_pack(cfg: Cfg, x_t: np.ndarray) -> np.ndarray:
    """[64, ET] float -> [64, ET//2] uint8; pairs columns (j, j+ST/2) of each
    supertile so that unpacked halves are contiguous on device."""
    ET = x_t.shape[1]
    step = cfg.q4_step
    q = np.clip(np.round(x_t / step + 7.5), 0, 15).astype(np.uint8)
    c = q.reshape(64, ET // cfg.ST, 2, cfg.ST // 2)
    return (c[:, :, 0, :] | (c[:, :, 1, :] << 4)).reshape(64, ET // 2)


def _prep(cfg: Cfg, inputs: dict, plan: Plan):
    h = np.asarray(inputs["h"], np.float32)
    eh = np.asarray(inputs["eh"], np.float32)
    src = np.asarray(inputs["src"]).astype(np.int64)
    dst = np.asarray(inputs["dst"]).astype(np.int64)
    W_node1 = np.asarray(inputs["W_node1"], np.float32)
    b_node1 = np.asarray(inputs["b_node1"], np.float32)
    W_node2 = np.asarray(inputs["W_node2"], np.float32)
    W_edge1 = np.asarray(inputs["W_edge1"], np.float32)
    b_edge1 = np.asarray(inputs["b_edge1"], np.float32)
    W_edge2 = np.asarray(inputs["W_edge2"], np.float32)
    W_comb = np.asarray(inputs["W_comb"], np.float32)
    W_ue = np.asarray(inputs["W_ue"], np.float32)

    NR, NW, ET, NC = cfg.NR, cfg.NW, plan.ET, cfg.NC
    shapes = _in_shapes(cfg, plan)
    glo = {name: np.zeros((NC * shp[0], *shp[1:]), dt)
           for name, (shp, dt) in shapes.items()}

    def per_core(name, k):
        d0 = shapes[name][0][0]
        return glo[name][k * d0:(k + 1) * d0]

    # folded weights (replicated to every core)
    A = (0.8 * W_edge1).astype(BF16)                 # [64, H]
    W_ue1 = (0.2 * (W_ue @ W_edge1)).astype(BF16)    # [64, H]
    wzp = np.concatenate([W_ue1, A], axis=0)         # [128, H]; rows 0:64 act on p
    iota_t = np.broadcast_to(np.arange(128, dtype=np.float32),
                             (128, 128)).astype(BF16)
    for k in range(NC):
        per_core("wzp", k)[:] = wzp
        per_core("we2", k)[:] = W_edge2.astype(BF16)
        per_core("wcomb", k)[:] = W_comb.astype(BF16)
        per_core("be1", k)[:] = b_edge1.reshape(cfg.H, 1)
        per_core("iota_t", k)[:] = iota_t
        per_core("wn1", k)[:] = W_node1.astype(BF16)
        per_core("wn2", k)[:] = W_node2.astype(BF16)
        per_core("bn1", k)[:] = b_node1.reshape(cfg.H, 1)

    hs_f8 = h.astype(F8)
    eh_f8 = eh.astype(F8)

    core = dst // NR
    win = (dst % NR) // 128

    for k in range(NC):
        # fill positions: bucket edges then pads
        perm = np.full(ET, -1, np.int64)
        ek = np.nonzero(core == k)[0]
        key = win[ek]
        order = np.argsort(key, kind="stable")
        ek = ek[order]
        key = key[order]
        starts = plan.pos0[key]
        changes = np.r_[True, key[1:] != key[:-1]]
        grp_start_idx = np.r_[0, np.nonzero(changes)[0][1:]]
        grp_id = np.cumsum(changes) - 1
        rank = np.arange(len(ek)) - grp_start_idx[grp_id]
        pos = starts + rank
        perm[pos] = ek

        valid = perm >= 0
        pe = perm[valid]

        if cfg.q4_eh:
            ehf = np.zeros((64, ET), np.float32)
            ehf[:, valid] = eh[pe].T
            per_core("eh_t", k)[:] = _q4_pack(cfg, ehf)
        else:
            per_core("eh_t", k)[:, valid] = eh_f8[pe].T
        if cfg.q4_hs:
            hsf = np.zeros((64, ET), np.float32)
            hsf[:, valid] = h[src[pe]].T
            per_core("hs_t", k)[:] = _q4_pack(cfg, hsf)
        else:
            per_core("hs_t", k)[:, valid] = hs_f8[src[pe]].T

        wrel = np.full(ET, PADVAL, np.uint8)
        wrel[valid] = (dst[pe] - k * NR - win[pe] * 128).astype(np.uint8)
        per_core("wrel_col", k)[:] = wrel.reshape(ET // 128, 128).T

        hwf = np.zeros((128, NW * 64), np.float32)
        hk = h[k * NR:(k + 1) * NR]                       # [NR, 64]
        for w in range(NW):
            rows = hk[w * 128:(w + 1) * 128]
            hwf[:rows.shape[0], w * 64:w * 64 + 64] = rows
        # int4-pack pairing global column halves (c, c + NW*32)
        step = cfg.q4_step
        q = np.clip(np.round(hwf / step + 7.5), 0, 15).astype(np.uint8)
        per_core("hwin", k)[:] = q[:, :NW * 32] | (q[:, NW * 32:] << 4)

    ctx = {"h": h}
    return glo, ctx


# --------------------------------------------------------------------------
# device program
# --------------------------------------------------------------------------

def _build(cfg: Cfg, plan: Plan) -> bacc.Bacc:
    ET, NW = plan.ET, cfg.NW
    f32 = mybir.dt.float32
    bf16 = mybir.dt.bfloat16
    f8 = mybir.dt.float8e3
    u8 = mybir.dt.uint8

    nc = bacc.Bacc("TRN2", target_bir_lowering=False, debug=False,
                   enable_asserts=False)

    if cfg.q4_eh:
        d_eh = nc.dram_tensor("eh_t", [64, ET // 2], u8, kind="ExternalInput").ap()
    else:
        d_eh = nc.dram_tensor("eh_t", [64, ET], f8, kind="ExternalInput").ap()
    if cfg.q4_hs:
        d_hst = nc.dram_tensor("hs_t", [64, ET // 2], u8, kind="ExternalInput").ap()
    else:
        d_hst = nc.dram_tensor("hs_t", [64, ET], f8, kind="ExternalInput").ap()
    d_wrc = nc.dram_tensor("wrel_col", [128, ET // 128], u8, kind="ExternalInput").ap()
    d_hwin = nc.dram_tensor("hwin", [128, NW * 32], u8, kind="ExternalInput").ap()
    d_wzp = nc.dram_tensor("wzp", [128, cfg.H], bf16, kind="ExternalInput").ap()
    d_we2 = nc.dram_tensor("we2", [cfg.H, cfg.H], bf16, kind="ExternalInput").ap()
    d_wcomb = nc.dram_tensor("wcomb", [cfg.H, 64], bf16, kind="ExternalInput").ap()
    d_be1 = nc.dram_tensor("be1", [cfg.H, 1], f32, kind="ExternalInput").ap()
    d_iota_t = nc.dram_tensor("iota_t", [128, 128], bf16, kind="ExternalInput").ap()
    d_wn1 = nc.dram_tensor("wn1", [64, cfg.H], bf16, kind="ExternalInput").ap()
    d_wn2 = nc.dram_tensor("wn2", [cfg.H, cfg.H], bf16, kind="ExternalInput").ap()
    d_bn1 = nc.dram_tensor("bn1", [cfg.H, 1], f32, kind="ExternalInput").ap()
    d_agg = nc.dram_tensor("agg", [128, NW * 64], f8, kind="ExternalOutput").ap()

    eq = mybir.AluOpType.is_equal
    mul = mybir.AluOpType.mult
    add = mybir.AluOpType.add
    Relu = mybir.ActivationFunctionType.Relu
    Tanh = mybir.ActivationFunctionType.Tanh

    NSTEP = ET // cfg.ST

    with tile.TileContext(nc) as tc, ExitStack() as ctx:
        con = ctx.enter_context(tc.tile_pool(name="const", bufs=1))
        sb = ctx.enter_context(tc.tile_pool(name="sb", bufs=2))
        sohp = ctx.enter_context(tc.tile_pool(name="soh", bufs=12))
        gpool = ctx.enter_context(tc.tile_pool(name="gbuf", bufs=2))
        pers = ctx.enter_context(tc.tile_pool(name="pers", bufs=1))
        ps_a = ctx.enter_context(tc.tile_pool(name="ps_a", bufs=1, space="PSUM"))
        ps_b = ctx.enter_context(tc.tile_pool(name="ps_b", bufs=1, space="PSUM"))
        ps_hd = ctx.enter_context(tc.tile_pool(name="ps_hd", bufs=1, space="PSUM"))
        ps_mn = ctx.enter_context(tc.tile_pool(name="ps_mn", bufs=1, space="PSUM"))
        ps_ag = ctx.enter_context(tc.tile_pool(name="ps_ag", bufs=1, space="PSUM"))

        def load_const(tag, dram_ap, shape, dtype):
            t_ = con.tile(shape, dtype, tag=tag)
            nc.sync.dma_start(out=t_[:], in_=dram_ap)
            return t_

        c_wzp = load_const("wzp", d_wzp, [128, cfg.H], bf16)
        c_we2 = load_const("we2", d_we2, [cfg.H, cfg.H], bf16)
        c_wcomb = load_const("wcomb", d_wcomb, [cfg.H, 64], bf16)
        c_be1 = load_const("be1", d_be1, [cfg.H, 1], f32)
        c_iota_t = load_const("iota_t", d_iota_t, [128, 128], bf16)
        c_hwin8 = load_const("hwin8", d_hwin, [128, NW * 32], u8)
        c_wrc8 = load_const("wrc8", d_wrc, [128, ET // 128], u8)
        c_wn1 = load_const("wn1", d_wn1, [64, cfg.H], bf16)
        c_wn2 = load_const("wn2", d_wn2, [cfg.H, cfg.H], bf16)
        c_bn1 = load_const("bn1", d_bn1, [cfg.H, 1], f32)

        # one-time uint8 -> f32 conversion of the per-chunk dst columns
        c_wrc = con.tile([128, ET // 128], f32, tag="wrc")
        nc.vector.tensor_copy(out=c_wrc[:], in_=c_wrc8[:])

        # one-time int4 -> bf16 decode of the dst-window node table
        dq_s = float(cfg.q4_step)
        dq_b = float(-7.5 * cfg.q4_step)
        c_hwn = con.tile([128, 2, NW * 32], u8, tag="hwn")
        nc.vector.tensor_scalar(c_hwn[:, 0, :], c_hwin8[:], 15, None,
                                mybir.AluOpType.bitwise_and)
        nc.vector.tensor_scalar(c_hwn[:, 1, :], c_hwin8[:], 4, None,
                                mybir.AluOpType.logical_shift_right)
        c_hwin = con.tile([128, NW * 64], bf16, tag="hwin")
        nc.scalar.activation(c_hwin[:, 0:NW * 32], c_hwn[:, 0, :],
                             mybir.ActivationFunctionType.Copy,
                             bias=dq_b, scale=dq_s)
        nc.scalar.activation(c_hwin[:, NW * 32:NW * 64], c_hwn[:, 1, :],
                             mybir.ActivationFunctionType.Copy,
                             bias=dq_b, scale=dq_s)

        agg_sb = pers.tile([128, NW * 64], f8)
        aggp = ps_ag.tile([128, 8, 64], f32)  # rotating window accumulators

        band = mybir.AluOpType.bitwise_and
        lsr = mybir.AluOpType.logical_shift_right
        q4s, q4b = float(cfg.q4_step), float(-7.5 * cfg.q4_step)
        HT = cfg.ST // 2

        for t in range(NSTEP):
            hsb = gpool.tile([64, cfg.ST], bf16, tag="hsb")
            if cfg.q4_hs:
                hsp = gpool.tile([64, HT], u8, tag="hsp")
                nc.sync.dma_start(out=hsp[:],
                                  in_=d_hst[:, t * HT:(t + 1) * HT])
                hsn = gpool.tile([64, 2, HT], u8, tag="hsn")
                nc.vector.tensor_scalar(hsn[:, 0, :], hsp[:], 15, None, band)
                nc.vector.tensor_scalar(hsn[:, 1, :], hsp[:], 4, None, lsr)
                nc.vector.tensor_scalar(hsb[:, 0:HT], hsn[:, 0, :],
                                        q4s, q4b, mul, add)
                nc.vector.tensor_scalar(hsb[:, HT:cfg.ST], hsn[:, 1, :],
                                        q4s, q4b, mul, add)
            else:
                hsq = gpool.tile([64, cfg.ST], f8, tag="hsq")
                nc.sync.dma_start(out=hsq[:],
                                  in_=d_hst[:, t * cfg.ST:(t + 1) * cfg.ST])
                nc.vector.tensor_copy(out=hsb[:], in_=hsq[:])

            # per-edge MLP1: m1 = relu(hs@Wn1 + bn1)@Wn2, in transposed form
            z1 = ps_a.tile([128, cfg.ST], f32, tag="za")
            for hhalf in range(cfg.ST // 512):
                cl0 = hhalf * 512
                nc.tensor.matmul(z1[:, cl0:cl0 + 512], c_wn1[:],
                                 hsb[:, cl0:cl0 + 512],
                                 start=True, stop=True)
            r1 = sb.tile([128, cfg.ST], bf16, tag="r1")
            nc.vector.tensor_scalar(r1[:], z1[:], c_bn1[:, 0:1], 0.0,
                                    mybir.AluOpType.add, mybir.AluOpType.max)
            m1p = ps_b.tile([128, cfg.ST], f32, tag="zb")
            for hhalf in range(cfg.ST // 512):
                cl0 = hhalf * 512
                nc.tensor.matmul(m1p[:, cl0:cl0 + 512], c_wn2[:],
                                 r1[:, cl0:cl0 + 512], start=True, stop=True)
            m1sb = sb.tile([128, cfg.ST], bf16, tag="m1sb")
            nc.vector.tensor_copy(out=m1sb[:], in_=m1p[:])

            stack = sb.tile([128, cfg.ST], bf16, tag="stack")
            if cfg.q4_eh:
                ehp = gpool.tile([64, HT], u8, tag="ehp")
                nc.sync.dma_start(out=ehp[:],
                                  in_=d_eh[:, t * HT:(t + 1) * HT])
                ehn = gpool.tile([64, 2, HT], u8, tag="ehn")
                nc.vector.tensor_scalar(ehn[:, 0, :], ehp[:], 15, None, band)
                nc.vector.tensor_scalar(ehn[:, 1, :], ehp[:], 4, None, lsr)
                nc.scalar.activation(stack[64:128, 0:HT], ehn[:, 0, :],
                                     mybir.ActivationFunctionType.Copy,
                                     bias=q4b, scale=q4s)
                nc.scalar.activation(stack[64:128, HT:cfg.ST], ehn[:, 1, :],
                                     mybir.ActivationFunctionType.Copy,
                                     bias=q4b, scale=q4s)
            else:
                ehq = gpool.tile([64, cfg.ST], f8, tag="ehq")
                nc.sync.dma_start(out=ehq[:],
                                  in_=d_eh[:, t * cfg.ST:(t + 1) * cfg.ST])
                nc.scalar.activation(stack[64:128, :], ehq[:],
                                     mybir.ActivationFunctionType.Copy)

            # per-128-chunk segment one-hot [edge, node-in-window]
            seg_ohs = []
            for j in range(cfg.ST // 128):
                c = t * (cfg.ST // 128) + j
                so = sohp.tile([128, 128], bf16, tag="soh")
                nc.vector.tensor_scalar(so[:], c_iota_t[:], c_wrc[:, c:c + 1],
                                        None, eq)
                seg_ohs.append(so)

            # hd via one-hot matmul, in 512-col halves; ohT = transpose(soh)
            for hhalf in range(cfg.ST // 512):
                cl0 = hhalf * 512
                ohT = sb.tile([128, 512], bf16, tag="ohT")
                for j in range(4):
                    nc.sync.dma_start_transpose(
                        out=ohT[:, j * 128:(j + 1) * 128],
                        in_=seg_ohs[cl0 // 128 + j][:])
                hd = ps_hd.tile([64, 512], f32, tag="hd")
                # window-parts inside this half (chunks are window-pure)
                j0 = cl0 // 128
                parts = []
                for j in range(j0, j0 + 4):
                    c = t * (cfg.ST // 128) + j
                    w = int(plan.wchunk[c])
                    if parts and parts[-1][2] == w:
                        parts[-1][1] += 128
                    else:
                        parts.append([j * 128 - cl0, 128, w])
                for (o, wd, w) in parts:
                    nc.tensor.matmul(hd[:, o:o + wd],
                                     c_hwin[:, w * 64:(w + 1) * 64],
                                     ohT[:, o:o + wd], start=True, stop=True)
                # p = hs * hd  -> stack partitions 0:64
                nc.vector.tensor_tensor(
                    out=stack[0:64, cl0:cl0 + 512],
                    in0=hsb[:, cl0:cl0 + 512],
                    in1=hd[:, :], op=mul)

            z = ps_a.tile([128, cfg.ST], f32, tag="za")
            for hhalf in range(cfg.ST // 512):
                cl0 = hhalf * 512
                nc.tensor.matmul(z[:, cl0:cl0 + 512], c_wzp[:],
                                 stack[:, cl0:cl0 + 512], start=True, stop=True)
            rz = sb.tile([128, cfg.ST], bf16, tag="rz")
            nc.scalar.activation(rz[:], z[:], Relu, bias=c_be1[:, 0:1])

            m2 = ps_b.tile([128, cfg.ST], f32, tag="zb")
            for hhalf in range(cfg.ST // 512):
                cl0 = hhalf * 512
                nc.tensor.matmul(m2[:, cl0:cl0 + 512], c_we2[:],
                                 rz[:, cl0:cl0 + 512], start=True, stop=True)

            m2c = sb.tile([128, cfg.ST], bf16, tag="m2c")
            nc.scalar.activation(m2c[:], m2[:],
                                 mybir.ActivationFunctionType.Copy)
            q = sb.tile([128, cfg.ST], bf16, tag="q")
            nc.gpsimd.tensor_tensor(out=q[:, :], in0=m1sb[:, :],
                                    in1=m2c[:, :], op=mul)

            mnt = ps_mn.tile([128, cfg.ST // 128, 64], f32, tag="mnt")
            for j in range(cfg.ST // 128):
                nc.tensor.matmul(mnt[:, j, :], q[:, j * 128:(j + 1) * 128],
                                 c_wcomb[:], start=True, stop=True)
            msb = sb.tile([128, cfg.ST // 128, 64], bf16, tag="msb")
            nc.scalar.activation(msb[:], mnt[:], Tanh)

            for j in range(cfg.ST // 128):
                c = t * (cfg.ST // 128) + j
                w = int(plan.wchunk[c])
                first = bool(plan.first_chunk[c])
                last = bool(plan.last_chunk[c])
                slot = w % 8
                nc.tensor.matmul(aggp[:, slot, :], seg_ohs[j][:],
                                 msb[:, j, :], start=first, stop=last)
                if last:
                    nc.vector.tensor_copy(out=agg_sb[:, w * 64:(w + 1) * 64],
                                          in_=aggp[:, slot, :])

        nc.sync.dma_start(out=d_agg, in_=agg_sb[:])

    nc.compile()
    return nc


# --------------------------------------------------------------------------
# cached PJRT executor (jit built once; zeros created device-side)
# --------------------------------------------------------------------------

class Exec:
    def __init__(self, nc_bass, n_cores: int):
        import jax
        import numpy as _np
        from jax.sharding import Mesh, NamedSharding, PartitionSpec
        from jax.experimental.shard_map import shard_map
        from concourse import bass2jax

        bass2jax.install_neuronx_cc_hook()
        self.nc = nc_bass
        self.n_cores = n_cores
        assert nc_bass.dbg_addr is None or not nc_bass.dbg_callbacks
        partition_name = (nc_bass.partition_id_tensor.name
                          if nc_bass.partition_id_tensor else None)

        in_names, out_names, out_avals, zero_shapes = [], [], [], []
        for alloc in nc_bass.m.functions[0].allocations:
            if not isinstance(alloc, mybir.MemoryLocationSet):
                continue
            name = alloc.memorylocations[0].name
            if alloc.kind == "ExternalInput":
                if name != partition_name:
                    in_names.append(name)
            elif alloc.kind == "ExternalOutput":
                out_names.append(name)
                shape = tuple(alloc.tensor_shape)
                dtype = mybir.dt.np(alloc.dtype)
                out_avals.append(jax.core.ShapedArray(shape, dtype))
                zero_shapes.append((shape, dtype))
        self.in_names = in_names
        self.out_names = out_names
        n_params = len(in_names)
        n_outs = len(out_avals)
        all_in_names = list(in_names) + list(out_names)
        if partition_name is not None:
            all_in_names.append(partition_name)
        donate = tuple(range(n_params, n_params + n_outs))
        self.zero_shapes = zero_shapes

        def _body(*args):
            operands = list(args)
            if partition_name is not None:
                operands.append(bass2jax.partition_id_tensor())
            outs = bass2jax._bass_exec_p.bind(
                *operands,
                out_avals=tuple(out_avals),
                in_names=tuple(all_in_names),
                out_names=tuple(out_names),
                lowering_input_output_aliases=(),
                sim_require_finite=True,
                sim_require_nnan=True,
                nc=nc_bass,
            )
            return tuple(outs)

        devices = jax.devices()[:n_cores]
        assert len(devices) == n_cores
        mesh = Mesh(_np.asarray(devices), ("core",))
        in_specs = (PartitionSpec("core"),) * (n_params + n_outs)
        out_specs = (PartitionSpec("core"),) * len(out_names)
        self.sharded = jax.jit(
            shard_map(_body, mesh=mesh, in_specs=in_specs,
                      out_specs=out_specs, check_rep=False),
            donate_argnums=donate, keep_unused=True,
        )
        self.sharding = NamedSharding(mesh, PartitionSpec("core"))
        import jax.numpy as jnp
        self._jnp = jnp
        self._donate_next = None

    def __call__(self, global_in: dict) -> dict:
        """global_in: name -> concatenated [n_cores*d0, ...] host array.
        Returns name -> concatenated output array (numpy)."""
        jnp = self._jnp
        args = [global_in[n] for n in self.in_names]
        if self._donate_next is None:
            # every output element is rewritten by the kernel, so any stale
            # device buffer works as the donated output slot; only the first
            # call has to materialize one (device-side fill, no transfer)
            outbufs = [jnp.zeros((self.n_cores * s[0], *s[1:]), d,
                                 device=self.sharding)
                       for (s, d) in self.zero_shapes]
        else:
            outbufs = self._donate_next
        outs = self.sharded(*args, *outbufs)
        host = {name: np.asarray(o) for name, o in zip(self.out_names, outs)}
        self._donate_next = list(outs)
        return host


# --------------------------------------------------------------------------
# entry points
# --------------------------------------------------------------------------

def _assemble(cfg: Cfg, agg_global: np.ndarray, ctx):
    h = ctx["h"]
    out = np.empty((cfg.N, cfg.DN), np.float32)
    for k in range(cfg.NC):
        agg = agg_global[k * 128:(k + 1) * 128].astype(np.float32)
        agg = agg.reshape(128, cfg.NW, 64).transpose(1, 0, 2).reshape(cfg.NW * 128, 64)
        out[k * cfg.NR:(k + 1) * cfg.NR] = agg[:cfg.NR] + h[k * cfg.NR:(k + 1) * cfg.NR]
    return out


def run_pipeline(cfg: Cfg, inputs: dict, backend: str = "hw", reps: int = 1):
    """Returns (output, steady_state_wall_seconds_or_None)."""
    src = np.asarray(inputs["src"]).astype(np.int64)
    dst = np.asarray(inputs["dst"]).astype(np.int64)
    plan = _make_plan(cfg, src, dst)
    glo, ctx = _prep(cfg, inputs, plan)
    nc = _build(cfg, plan)
    if backend == "sim":
        from concourse.bass_interp import CoreSim
        shapes = _in_shapes(cfg, plan)
        aggs = []
        for k in range(cfg.NC):
            sim = CoreSim(nc, trace=False)
            for name, (shp, _dt) in shapes.items():
                d0 = shp[0]
                sim.tensor(name)[:] = glo[name][k * d0:(k + 1) * d0]
            sim.simulate()
            aggs.append(np.array(sim.tensor("agg")))
        return _assemble(cfg, np.concatenate(aggs, axis=0), ctx), None
    ex = Exec(nc, cfg.NC)
    res = ex(glo)  # first call traces + compiles
    dt = None
    for _ in range(max(0, reps - 1)):
        t0 = time.time()
        res = ex(glo)
        out = _assemble(cfg, res["agg"], ctx)
        dt = time.time() - t0
    out = _assemble(cfg, res["agg"], ctx)
    return out, dt


def kernel(**inputs) -> np.ndarray:
    out, _ = run_pipeline(CFG_FULL, inputs, backend="hw")
    return out


if __name__ == "__main__":
    # smoke test at small scale on the simulator
    cfg = Cfg(N=2048, E=8192, NC=2, ST=1024, q4_eh=True, q4_hs=True)
    rng = np.random.default_rng(0)
    inputs = {
        "h": rng.standard_normal((cfg.N, 64)).astype(np.float32),
        "eh": rng.standard_normal((cfg.E, 64)).astype(np.float32),
        "W_node1": (rng.standard_normal((64, 128)) * 0.05).astype(np.float32),
        "b_node1": (rng.standard_normal((128,)) * 0.05).astype(np.float32),
        "W_node2": (rng.standard_normal((128, 128)) * 0.05).astype(np.float32),
        "W_edge1": (rng.standard_normal((64, 128)) * 0.05).astype(np.float32),
        "b_edge1": (rng.standard_normal((128,)) * 0.05).astype(np.float32),
        "W_edge2": (rng.standard_normal((128, 128)) * 0.05).astype(np.float32),
        "W_comb": (rng.standard_normal((128, 64)) * 0.05).astype(np.float32),
        "W_ue": (rng.standard_normal((64, 64)) * 0.05).astype(np.float32),
        "src": rng.integers(0, cfg.N, cfg.E).astype(np.int32),
        "dst": rng.integers(0, cfg.N, cfg.E).astype(np.int32),
    }
    h, eh = inputs["h"], inputs["eh"]
    hs, hd = h[inputs["src"]], h[inputs["dst"]]
    eh_new = 0.8 * eh + 0.2 * ((hs * hd) @ inputs["W_ue"])
    m1 = np.maximum(hs @ inputs["W_node1"] + inputs["b_node1"], 0) @ inputs["W_node2"]
    m2 = np.maximum(eh_new @ inputs["W_edge1"] + inputs["b_edge1"], 0) @ inputs["W_edge2"]
    m = np.tanh((m1 * m2) @ inputs["W_comb"])
    agg = np.zeros((cfg.N, 64), np.float32)
    np.add.at(agg, inputs["dst"], m)
    expected = agg + h

    out, _ = run_pipeline(cfg, inputs, backend="sim")
    err = np.abs(out - expected)
    rel = np.abs(err).max() / np.abs(expected).max()
    print("max abs err:", err.max(), " rel(absmax):", rel)
    print("mean abs err:", err.mean())
    assert rel < 2e-2, "accuracy failure"
    print("SIM OK")
